# revision 8
# baseline (speedup 1.0000x reference)
"""LLRNN Trainium2 kernel.

Reference computation (per batch row b):
    zx[b,t,:] = x[b,t,:] @ K + bias          (K: [64, 256], bias: [256])
    c_t = f * c_{t-1} + (1-f) * tanh(z_i)    where
        z = zx_t + c_{t-1} @ R               (R: [128, 256])
        f = sigmoid(z[:, :128]), z_i = z[:, 128:]
    out[b,:] = c_T @ Wd + bd                 (Wd: [128, 64], bd: [64])

Mapping (per core, batch shard of 512):
  - State kept transposed: c [128 cell partitions, batch free], fp16 working
    copy + fp32 shadow accumulated via DMA (off the critical path), resynced
    into the working copy every RESYNC steps. Update in delta form:
        s = sigmoid(-z_f) = 1-f;  e = g - c16;  d = s*e
        c16 += d (DVE);  c32 += d (DMA accumulate)
    which keeps all per-step DVE ops in fp16 2x mode while the integrated
    state error stays ~1e-4 (fp16 state alone random-walks to ~1e-2).
  - Host pre-transposes x to [t-pair, 128 (j*64+i), 512 b] fp16 so every DMA
    is a contiguous 128-partition 128KB tile; no on-device transposes.
  - Input-projection weights zero-padded to K=128 (row block j*64..j*64+64
    holds K for timestep parity j) so FWL stays enabled and the rhs is the
    full 128-partition x tile for both timesteps of a pair.
  - Gate bias applied via the ACT per-partition bias operand.
  - Two independent half-batch chains (A/B, 256 cols each) pipeline across
    PE (matmuls), ACT (sigmoid), DVE (blend).
  - Final dense layer: c32 chunks become the matmul stationary so the output
    lands batch-major in PSUM; bias added via DVE with a broadcast tile.
"""

import sys

sys.path.insert(0, "/opt/trn_rl_repo")

import numpy as np

T_FULL = 256
IN_DIMS = 64
CELL = 128
OUT_DIMS = 64
BATCH = 4096
NCORES = 8
BC = BATCH // NCORES  # 512 batch rows per core
HB = BC // 2  # 256: half-batch chain width
RESYNC = 8  # steps between c16 <- c32 refreshes


def build_nc(T=T_FULL):
    import concourse.bass as bass
    import concourse.tile as tile
    from concourse import bacc, mybir
    from contextlib import ExitStack

    f16 = mybir.dt.float16
    f32 = mybir.dt.float32
    AF = mybir.ActivationFunctionType
    OP = mybir.AluOpType

    nc = bacc.Bacc("TRN2", target_bir_lowering=False, debug=False)

    ntp = T // 2
    xp_d = nc.dram_tensor("xp", [ntp, 128, BC], f16, kind="ExternalInput")
    # [128, 512]: cols j*256..j*256+256 = gates (f|i) for timestep parity j,
    # rows j*64..j*64+64 = K rows (other rows zero)
    wk_d = nc.dram_tensor("wk", [128, 512], f16, kind="ExternalInput")
    wr_d = nc.dram_tensor("wr", [CELL, 2 * CELL], f16, kind="ExternalInput")
    nbf_d = nc.dram_tensor("nbf", [CELL, 1], f32, kind="ExternalInput")  # -bias_f
    bi_d = nc.dram_tensor("bi", [CELL, 1], f32, kind="ExternalInput")
    dw_d = nc.dram_tensor("dw", [CELL, OUT_DIMS], f32, kind="ExternalInput")
    db_d = nc.dram_tensor("db", [OUT_DIMS], f32, kind="ExternalInput")
    out_d = nc.dram_tensor("out", [BC, OUT_DIMS], f32, kind="ExternalOutput")

    with tile.TileContext(nc) as tc, ExitStack() as ctx:
        wpool = ctx.enter_context(tc.tile_pool(name="w", bufs=1))
        xpool = ctx.enter_context(tc.tile_pool(name="x", bufs=8))
        gpool = ctx.enter_context(tc.tile_pool(name="g", bufs=3))
        spool = ctx.enter_context(tc.tile_pool(name="s", bufs=1))
        zpool = ctx.enter_context(tc.tile_pool(name="z", bufs=2, space="PSUM"))

        # ---- constants / weights ----
        wk_sb = wpool.tile([128, 512], f16, tag="wk")
        nc.sync.dma_start(wk_sb[:], wk_d[:])
        wr_sb = wpool.tile([CELL, 2 * CELL], f16, tag="wr")
        nc.sync.dma_start(wr_sb[:], wr_d[:])
        nbf_sb = wpool.tile([CELL, 1], f32, tag="nbf")
        nc.sync.dma_start(nbf_sb[:], nbf_d[:])
        bi_sb = wpool.tile([CELL, 1], f32, tag="bi")
        nc.sync.dma_start(bi_sb[:], bi_d[:])
        dw_sb = wpool.tile([CELL, OUT_DIMS], f32, tag="dw")
        nc.sync.dma_start(dw_sb[:], dw_d[:])
        # dense bias broadcast across all 128 partitions
        db_sb = wpool.tile([128, OUT_DIMS], f32, tag="db")
        db_ap = db_d[:]
        db_bcast = bass.AP(
            tensor=db_ap.tensor, offset=db_ap.offset, ap=[[0, 128]] + list(db_ap.ap)
        )
        nc.sync.dma_start(db_sb[:], db_bcast)

        # ---- state (two half-batch chains) ----
        cs = [spool.tile([CELL, HB], f16, tag=f"c{h}", name=f"c{h}") for h in range(2)]
        c32s = [
            spool.tile([CELL, HB], f32, tag=f"c32{h}", name=f"c32{h}") for h in range(2)
        ]
        for c in cs:
            nc.vector.memset(c[:], 0.0)
        for c in c32s:
            nc.vector.memset(c[:], 0.0)

        wkp = [  # [j][gate] zero-padded input-proj weights
            [wk_sb[:, 0:128], wk_sb[:, 128:256]],
            [wk_sb[:, 256:384], wk_sb[:, 384:512]],
        ]
        wrf = wr_sb[:, 0:CELL]
        wri = wr_sb[:, CELL : 2 * CELL]

        # ---- recurrence ----
        for tp in range(ntp):
            xt = xpool.tile([128, BC], f16, tag="xt")
            nc.sync.dma_start(xt[:], xp_d[tp])
            for j in range(2):
                t = 2 * tp + j
                zf = [
                    zpool.tile([CELL, HB], f32, tag=f"zf{h}", name=f"zf{h}_{t}")
                    for h in range(2)
                ]
                zi = [
                    zpool.tile([CELL, HB], f32, tag=f"zi{h}", name=f"zi{h}_{t}")
                    for h in range(2)
                ]
                # input projections (state-independent; K padded to 128 so the
                # rhs is the same full x tile for both parities)
                for h in range(2):
                    xh = xt[:, h * HB : (h + 1) * HB]
                    nc.tensor.matmul(zf[h][:], wkp[j][0], xh, start=True, stop=False)
                for h in range(2):
                    xh = xt[:, h * HB : (h + 1) * HB]
                    nc.tensor.matmul(zi[h][:], wkp[j][1], xh, start=True, stop=False)
                # recurrent matmuls, same stationary back-to-back
                for h in range(2):
                    nc.tensor.matmul(zf[h][:], wrf, cs[h][:], start=False, stop=True)
                for h in range(2):
                    nc.tensor.matmul(zi[h][:], wri, cs[h][:], start=False, stop=True)
                for h in range(2):
                    s = gpool.tile([CELL, HB], f16, tag=f"s{h}")
                    nc.scalar.activation(
                        s[:], zf[h][:], AF.Sigmoid, bias=nbf_sb[:], scale=-1.0
                    )
                    g = gpool.tile([CELL, HB], f16, tag=f"g{h}")
                    nc.scalar.activation(g[:], zi[h][:], AF.Tanh, bias=bi_sb[:])
                    e = gpool.tile([CELL, HB], f16, tag=f"e{h}")
                    nc.vector.tensor_tensor(e[:], g[:], cs[h][:], OP.subtract)
                    d = gpool.tile([CELL, HB], f16, tag=f"d{h}")
                    nc.vector.tensor_tensor(d[:], s[:], e[:], OP.mult)
                    nc.vector.tensor_tensor(cs[h][:], cs[h][:], d[:], OP.add)
                    # fp32 shadow accumulate, off the critical path
                    nc.gpsimd.dma_start(c32s[h][:], d[:], accum_op=OP.add)
                    if (t + 1) % RESYNC == 0 and t != T - 1:
                        nc.gpsimd.dma_start(cs[h][:], c32s[h][:])

        # ---- dense head: out[b,:] = c32_T[:,b] . dw + db ----
        for h in range(2):
            for k in range(HB // 128):
                ops = zpool.tile([128, OUT_DIMS], f32, tag="zf0")
                nc.tensor.matmul(
                    ops[:],
                    c32s[h][:, k * 128 : (k + 1) * 128],
                    dw_sb[:],
                    start=True,
                    stop=True,
                )
                osb = gpool.tile([128, OUT_DIMS], f32, tag="osb")
                nc.vector.tensor_tensor(osb[:], ops[:], db_sb[:], OP.add)
                b0 = h * HB + k * 128
                nc.sync.dma_start(out_d[b0 : b0 + 128, :], osb[:])

    nc.compile()
    return nc


def prep_host(inputs, kernel, recurrent_kernel, recurrent_bias, dense_w, dense_b,
              T=T_FULL):
    """Build per-core input maps (host-side layout transform + fp16 casts)."""
    x = np.asarray(inputs)
    B = x.shape[0]
    bc = B // NCORES
    wk1 = np.asarray(kernel, np.float16)  # [64, 256]
    wk = np.zeros((128, 512), np.float16)
    wk[0:64, 0:256] = wk1  # parity 0: K rows at partitions 0..64
    wk[64:128, 256:512] = wk1  # parity 1: K rows at partitions 64..128
    wr = np.ascontiguousarray(np.asarray(recurrent_kernel, np.float16))
    rb = np.asarray(recurrent_bias, np.float32)
    nbf = np.ascontiguousarray(-rb[:CELL].reshape(CELL, 1))
    bi = np.ascontiguousarray(rb[CELL:].reshape(CELL, 1))
    dw = np.ascontiguousarray(np.asarray(dense_w, np.float32))
    db = np.ascontiguousarray(np.asarray(dense_b, np.float32))

    x16 = x.astype(np.float16)  # [B, T, 64]
    in_maps = []
    for c in range(NCORES):
        xc = x16[c * bc : (c + 1) * bc]  # [bc, T, 64]
        xt = np.ascontiguousarray(xc.transpose(1, 2, 0))  # [T, 64, bc]
        xp = xt.reshape(T // 2, 128, bc)  # t-pair packing: p = j*64+i
        in_maps.append(
            {"xp": xp, "wk": wk, "wr": wr, "nbf": nbf, "bi": bi, "dw": dw, "db": db}
        )
    return in_maps


_NC_CACHE = {}


def kernel(**inp):
    from concourse import bass_utils

    T = inp["inputs"].shape[1]
    if T not in _NC_CACHE:
        _NC_CACHE[T] = build_nc(T)
    nc = _NC_CACHE[T]
    in_maps = prep_host(**inp)
    res = bass_utils.run_bass_kernel_spmd(nc, in_maps, core_ids=list(range(NCORES)))
    out = np.concatenate([r["out"] for r in res.results], axis=0)
    return out.astype(np.float32)
